# revision 1
# baseline (speedup 1.0000x reference)
"""Trainium2 Bass kernel for nn_Blur2: depthwise 4x4 blur (upfirdn2d-style,
pad=(2,1), unit stride) over input [8, 128, 256, 256] f32.

Strategy: pure data parallel over the 1024 independent (n, c) planes --
128 planes per NeuronCore. Within a plane the 2D 16-tap conv runs on the
tensor engine as banded matmuls: the H-direction conv is the contraction
(banded Toeplitz fp16 weights, image rows on partitions) and the
W-direction conv is 4 shifted slices of the moving operand accumulated
into the same PSUM tile (clipped column ranges encode the zero padding,
clipped weight bands encode the H padding).

Precision: fp32 input is split on host (exact) into hi = fp16(x) and
lo = fp16(x - hi); blur weights (entries k/16) are exact in fp16; fp16
products accumulate exactly in fp32 PSUM -> max rel err ~2e-7.

DMA: planes are packed in QUADS on host -- each DRAM row holds 4 planes'
(hi|lo) data = 4 KB -- so every DMA moves >=4KB per partition, which is
the difference between ~100 GB/s and ~340 GB/s per core on TRN2. The
output uses a 260-row-per-quad DRAM layout (junk rows at 127, 253..255)
so both store DMAs are exactly 128 partitions: the HWDGE splits an
SBUF->DRAM DMA across its 16 SDMA engines only when the partition count
divides into 16 chunks (128p -> 16 engines / 381 GB/s; 127p -> 1 engine
/ 25 GB/s). Loads go on the sync HWDGE ring, stores on the scalar HWDGE
ring (separate queues so store emission never stalls load prefetch);
the gpsimd SWDGE path is avoided for bulk data (it emits ~8 tiny
semaphore packets per data packet, capping at ~130 GB/s).

Measured on 8 cores: HW exec ~250 us (PE ~93% busy at its 109 ns/matmul
streaming floor; ~20 junk warmup matmuls lift the HAM clock gate to
2.4 GHz during the DMA ramp so the real stream starts warm), max rel
err 2.2e-7 vs the fp32 jax reference.
"""
import sys

for _p in ("/opt/trn_rl_repo", "/opt/pypackages"):
    if _p not in sys.path:
        sys.path.insert(0, _p)

import contextlib

import numpy as np


def _install_ntff_hook_shim():
    """The agent image's antenv lacks axon_hooks, which bass_utils needs
    for trace=True under axon. Provide it in sys.modules, backed by
    trn_agent_boot's ctypes NTFF shim."""
    import types

    if "antenv.axon_hooks" in sys.modules:
        return
    mod = types.ModuleType("antenv.axon_hooks")
    state = {"hook": None, "tried": False}

    def set_axon_ntff_profile_hook(hook):
        state["hook"] = hook

    def get_axon_ntff_profile_hook():
        if state["hook"] is None and not state["tried"]:
            state["tried"] = True
            try:
                from trn_agent_boot.trn_boot import _ntff_profile_via_ctypes

                state["hook"] = _ntff_profile_via_ctypes("/opt/axon/libaxon_pjrt.so")
            except Exception:
                state["hook"] = None
        return state["hook"]

    mod.set_axon_ntff_profile_hook = set_axon_ntff_profile_hook
    mod.get_axon_ntff_profile_hook = get_axon_ntff_profile_hook
    sys.modules["antenv.axon_hooks"] = mod
    try:
        import antenv

        antenv.axon_hooks = mod
    except ImportError:
        pass


_install_ntff_hook_shim()

import concourse.bacc as bacc
import concourse.tile as tile
from concourse import mybir
from concourse.bass_utils import run_bass_kernel_spmd

N_CORES = 8
H = W = 256
PLANES = 1024 // N_CORES  # 128 per core
Q = 4  # planes packed per SBUF/DRAM row
NQUAD = PLANES // Q  # 32 quad-groups per core
SEC = 2 * W  # one plane's section in a packed row: hi[0:256] | lo[256:512]

# M-tile layout along H per plane:
#   tile A: out rows [0, 127)   from x rows [0, 128)
#   tile B: out rows [127, 252) from x rows [125, 253)
#   remainder: out rows [252, 256) from x rows [250, 256), stacked across
#   groups of RG=16 quads (96 partitions, 64 out rows per plane-slot)
MA, MB = 127, 125
RG = 16

# per W-shift i: out cols [wl, wh), reading x cols [cl, ch)  (tap = w-2+i)
SHIFT_RANGES = {
    0: (2, 256, 0, 254),
    1: (1, 256, 0, 255),
    2: (0, 256, 0, 256),
    3: (0, 255, 1, 256),
}
SHIFT_ORDER = [2, 0, 1, 3]  # full-range shift first so start=True covers all


def _make_weights(wk: np.ndarray):
    """wk: flipped 4x4 kernel. Packed fp16 weights, one 128-col matrix per
    W-shift (cols padded with zeros past MA/MB so NumWeights==128 enables
    the PE Fast-Weight-Load path): wa/wb [128, 4*128], wr [96, 4*64]
    (block-diag 16x(6->4))."""
    wa = np.zeros((128, 4, 128), np.float32)
    for k in range(128):
        for m in range(MA):
            d = k - m + 2
            if 0 <= d <= 3:
                wa[k, :, m] = wk[d, :]
    wb = np.zeros((128, 4, 128), np.float32)
    for k in range(128):
        for m in range(MB):
            d = k - m
            if 0 <= d <= 3:
                wb[k, :, m] = wk[d, :]
    wr = np.zeros((RG * 6, 4, RG * 4), np.float32)
    for b in range(RG):
        for r in range(6):
            for c in range(4):
                d = r - c
                if 0 <= d <= 3:
                    wr[6 * b + r, :, 4 * b + c] = wk[d, :]
    return (
        wa.reshape(128, 4 * 128).astype(np.float16),
        wb.reshape(128, 4 * 128).astype(np.float16),
        wr.reshape(RG * 6, 4 * RG * 4).astype(np.float16),
    )


def _build_program(nquad: int = NQUAD):
    nc = bacc.Bacc("TRN2", target_bir_lowering=False, debug=False)
    f16, f32 = mybir.dt.float16, mybir.dt.float32

    d_xs = nc.dram_tensor("xs", [nquad, H, Q * SEC], f16, kind="ExternalInput").ap()
    d_wa = nc.dram_tensor("wa", [128, 4 * 128], f16, kind="ExternalInput").ap()
    d_wb = nc.dram_tensor("wb", [128, 4 * 128], f16, kind="ExternalInput").ap()
    d_wr = nc.dram_tensor("wr", [RG * 6, 4 * RG * 4], f16, kind="ExternalInput").ap()
    d_out = nc.dram_tensor("out", [nquad, H + 4, Q * W], f32, kind="ExternalOutput").ap()

    rem_groups = [(s, min(RG, nquad - s)) for s in range(0, nquad, RG)]

    with tile.TileContext(nc) as tc, contextlib.ExitStack() as ctx:
        wpool = ctx.enter_context(tc.tile_pool(name="wpool", bufs=1))
        xin = ctx.enter_context(tc.tile_pool(name="xin", bufs=6))
        xinr = ctx.enter_context(tc.tile_pool(name="xinr", bufs=2))
        psum = ctx.enter_context(tc.tile_pool(name="psum", bufs=2, space="PSUM"))
        outp = ctx.enter_context(tc.tile_pool(name="outp", bufs=6))
        outr = ctx.enter_context(tc.tile_pool(name="outr", bufs=2))

        # PE warmup: ~20 junk matmuls with no data dependencies, issued
        # before any real work. They run during the DMA ramp (t~5-10us)
        # and lift the HAM clock gate to 2.4 GHz before the real stream
        # starts. Results land in a scratch psum slot and are discarded;
        # any garbage/NaN is overwritten later because every bank's first
        # real matmul runs with start=True.
        warm = wpool.tile([128, W], f16, tag="warm")
        nc.vector.memset(warm[:], 0.0)
        psW = psum.tile([128, W], f32, tag="psA")
        for _ in range(20):
            nc.tensor.matmul(
                psW[:, :], warm[:, :128], warm[:, :],
                start=True, stop=True, skip_group_check=True,
            )

        t_wa = wpool.tile([128, 4 * 128], f16, tag="wa")
        nc.scalar.dma_start(out=t_wa[:], in_=d_wa)
        t_wb = wpool.tile([128, 4 * 128], f16, tag="wb")
        nc.scalar.dma_start(out=t_wb[:], in_=d_wb)
        t_wr = wpool.tile([RG * 6, 4 * RG * 4], f16, tag="wr")
        nc.scalar.dma_start(out=t_wr[:], in_=d_wr)

        def conv_mms(ps, wt, wcols, mrows, xt, xrows):
            """4 shifts x 4 quad-planes x hi/lo accumulating matmuls into the
            quad psum tile ps [mrows, Q*W] (2 banks; per-bank first mm gets
            start=True)."""
            last = (SHIFT_ORDER[-1], Q - 1, 1)
            for i in SHIFT_ORDER:
                wl, wh, cl, ch = SHIFT_RANGES[i]
                lhsT = wt[:xrows, i * 128 : i * 128 + 128]
                for q in range(Q):
                    for half in (0, 1):
                        mm = nc.tensor.matmul(
                            ps[:128, q * W + wl : q * W + wh],
                            lhsT,
                            xt[:xrows, q * SEC + half * W + cl : q * SEC + half * W + ch],
                            start=(i == SHIFT_ORDER[0] and half == 0 and q % 2 == 0),
                            stop=((i, q, half) == last),
                            skip_group_check=True,
                        )

        def emit_out(ps, mrows, dram_view, alt):
            o = outp.tile([128, Q * W], f32, tag="oa" if alt else "ob")
            hw = Q * W // 2
            if alt:
                nc.scalar.copy(o[:, :hw], ps[:, :hw])
                nc.vector.tensor_copy(o[:, hw:], ps[:, hw:])
            else:
                nc.vector.tensor_copy(o[:, :hw], ps[:, :hw])
                nc.scalar.copy(o[:, hw:], ps[:, hw:])
            nc.scalar.dma_start(out=dram_view, in_=o[:])

        ri = 0
        for g in range(nquad):
            ta = xin.tile([128, Q * SEC], f16, tag="ta")
            if g == 0:
                # split the very first load so the PE stream starts sooner
                nc.sync.dma_start(out=ta[0:64, :], in_=d_xs[g, 0:64, :])
                nc.sync.dma_start(out=ta[64:128, :], in_=d_xs[g, 64:128, :])
            else:
                nc.sync.dma_start(out=ta[:], in_=d_xs[g, 0:128, :])
            tb = xin.tile([128, Q * SEC], f16, tag="tb")
            nc.sync.dma_start(out=tb[:], in_=d_xs[g, 125:253, :])

            psA = psum.tile([128, Q * W], f32, tag="psA")
            conv_mms(psA, t_wa, MA, MA, ta, 128)
            emit_out(psA, MA, d_out[g, 0:128, :], alt=(g % 2 == 0))

            psB = psum.tile([128, Q * W], f32, tag="psB")
            conv_mms(psB, t_wb, MB, MB, tb, 128)
            emit_out(psB, MB, d_out[g, 128:256, :], alt=(g % 2 == 1))

            # stacked remainder: input rows come straight from DRAM, so
            # emit early (quads 2, 4, ...) to keep them off the kernel tail
            if ri < len(rem_groups) and g == min(2 * (ri + 1), nquad - 1):
                s, gsz = rem_groups[ri]
                ri += 1
                tr = xinr.tile([RG * 6, Q * SEC], f16, tag="tr")
                nc.sync.dma_start(
                    out=tr[: 6 * gsz, :], in_=d_xs[s : s + gsz, 250:256, :]
                )
                psR = psum.tile([RG * 4, Q * W], f32, tag="psA")
                last = (SHIFT_ORDER[-1], Q - 1, 1)
                for i in SHIFT_ORDER:
                    wl, wh, cl, ch = SHIFT_RANGES[i]
                    lhsT = t_wr[: 6 * gsz, i * RG * 4 : i * RG * 4 + 4 * gsz]
                    for q in range(Q):
                        for half in (0, 1):
                            mm = nc.tensor.matmul(
                                psR[: 4 * gsz, q * W + wl : q * W + wh],
                                lhsT,
                                tr[: 6 * gsz, q * SEC + half * W + cl : q * SEC + half * W + ch],
                                start=(i == SHIFT_ORDER[0] and half == 0 and q % 2 == 0),
                                stop=((i, q, half) == last),
                                skip_group_check=True,
                            )
                orr = outr.tile([RG * 4, Q * W], f32, tag="orr")
                if g % 2 == 0:
                    nc.scalar.copy(orr[: 4 * gsz, :], psR[: 4 * gsz, :])
                else:
                    nc.vector.tensor_copy(orr[: 4 * gsz, :], psR[: 4 * gsz, :])
                nc.scalar.dma_start(
                    out=d_out[s : s + gsz, H : H + 4, :], in_=orr[: 4 * gsz, :]
                )

    nc.compile()
    return nc


_CACHE = {}


def _get_program(nquad: int = NQUAD):
    if nquad not in _CACHE:
        _CACHE[nquad] = _build_program(nquad)
    return _CACHE[nquad]


def _run(x: np.ndarray, wk: np.ndarray, trace: bool = False):
    """x: [P, 256, 256] f32 full stack of planes (P divisible by 8*Q),
    wk: flipped 4x4 kernel. Returns ([P, 256, 256] f32, exec_time_ns|None)."""
    P = x.shape[0]
    qper = P // (N_CORES * Q)
    hi = x.astype(np.float16)
    lo = (x - hi.astype(np.float32)).astype(np.float16)
    xs = np.concatenate([hi, lo], axis=2)  # [P, 256, 512]
    # quad-pack: [P/Q, Q, H, SEC] -> [P/Q, H, Q, SEC] -> [P/Q, H, Q*SEC]
    xsq = (
        xs.reshape(P // Q, Q, H, SEC)
        .transpose(0, 2, 1, 3)
        .reshape(P // Q, H, Q * SEC)
    )

    wa, wb, wr = _make_weights(wk)
    nc = _get_program(qper)

    in_maps = [
        {
            "xs": np.ascontiguousarray(xsq[c * qper : (c + 1) * qper]),
            "wa": wa,
            "wb": wb,
            "wr": wr,
        }
        for c in range(N_CORES)
    ]
    res = run_bass_kernel_spmd(nc, in_maps, list(range(N_CORES)), trace=trace)
    outq = np.concatenate([r["out"] for r in res.results], axis=0)  # [P/Q, H+4, Q*W]
    outq = np.concatenate(
        [outq[:, 0:127], outq[:, 128:253], outq[:, 256:260]], axis=1
    )  # drop junk rows -> [P/Q, 256, Q*W]
    out = (
        outq.reshape(P // Q, H, Q, W)
        .transpose(0, 2, 1, 3)
        .reshape(P, H, W)
    )
    return np.ascontiguousarray(out), res.exec_time_ns


def kernel(input: np.ndarray, kernel: np.ndarray) -> np.ndarray:
    x = np.asarray(input, dtype=np.float32)
    k = np.asarray(kernel, dtype=np.float32)
    n, c, h, w = x.shape
    wk = np.flip(k, (0, 1)).copy()  # correlation weights
    out, _ = _run(x.reshape(n * c, h, w), wk, trace=False)
    return out.reshape(n, c, h, w)

